# revision 33
# baseline (speedup 1.0000x reference)
"""Trainium2 Bass kernel for the batched natural-cubic-spline + MLP model.

Math: the whole spline pipeline (natural-cubic coeffs via the constant
tridiagonal solve, evaluation at t = sigmoid(raw_index)) is linear in x:
    outputs = x @ E,   E (N x T),  col j = c0*e_i + c1*e_{i+1}
                                          + c2*K[:,i] + c3*K[:,i+1]
with K = R @ inv(Tridiag) input-independent. E depends only on raw_index,
so M1 = E @ W1 (N x 50) is computed ON HOST in f64 and shipped as bf16.

Device work per core (pure data-parallel, batch split 8 ways):
    h1 = leaky(x @ M1_bf16 + b1')    # 17 k-chunk matmuls x 2 col halves
    h2 = leaky(h1 @ W2 + b2); y = h2 @ W3 + b3
The kernel is HBM-bound, so x ships in mixed precision: the NBF bands
with the largest |M1| rows (biggest error impact) as bf16, the rest as
fp8 e4m3 recentered to x-0.5 (uniform [0,1) data => half the quant
error); the 0.5 shift is folded into b1' on host. Bands are permuted by
magnitude on host; contraction order is irrelevant to the matmul.
x^T streams over both HWDGE queues in host-packed GROUP tiles (small
head groups for an early PE start, big middle groups to amortize the
~0.7us inter-DMA queue bubble). Dummy matmuls at kernel start keep the
PE busy so its DVFS clock is ramped (2.4GHz) when the real stream begins.
Biases ride the scalar-engine activations (bias=), leaky via Lrelu.
"""

import functools

import numpy as np

N = 2000          # bands (spline knots)
T = 500           # eval points
BATCH = 8192
NCORES = 8
BPC = BATCH // NCORES      # 1024 batch rows per core
HID = 50
HID2 = 10
H = 1.0 / (N - 1)
NBF = 384         # bands kept in bf16 (largest |M1| rows)
# contraction chunks in PE order: tiny fp8 head (earliest PE start), then
# uniform ~256KB groups strictly alternating queue and interleaving the
# bf16 singles between fp8 pairs, so consumption tracks both queues
CHUNKS = [32, 96, 128, 128] + [128] + [128, 128] + [128] + [128, 128] + [128] + [128] * 4 + [128, 80]
CF8 = [True] * 4 + [False] + [True, True] + [False] + [True, True] + [False] + [True] * 6
COFF = np.concatenate([[0], np.cumsum(CHUNKS)]).astype(int)
KT = len(CHUNKS)  # 17
MW = KT * HID     # m1 packed width (850)
# x^T DMA groups: (first chunk, n chunks)
GROUPS = [
    (0, 1), (1, 1), (2, 2), (4, 1), (5, 2), (7, 1),
    (8, 2), (10, 1), (11, 2), (13, 2), (15, 2),
]
GROWS = [32, 96, 128, 128, 128, 128, 128, 128, 128, 128, 128]
GF8 = [True, True, True, False, True, False, True, False, True, True, True]
# queue per group (0 = sync, 1 = scalar), alternating in chunk order
GQ = [0, 1, 0, 1, 0, 1, 0, 1, 0, 1, 0]
M1SPLIT = 8       # m1 head covers chunks [0, M1SPLIT), tail the rest + W2/W3


# ----------------------------------------------------------------- host math
@functools.lru_cache(maxsize=1)
def _k_matrix():
    """K (N x N) f64 with kd = x @ K (knot derivatives)."""
    hr = float(N - 1)
    main = np.full(N, 4.0 * hr)
    main[0] = main[-1] = 2.0 * hr
    off = np.full(N - 1, hr)
    A = np.diag(main) + np.diag(off, 1) + np.diag(off, -1)
    A_inv = np.linalg.inv(A)
    R = np.zeros((N, N))
    c = 3.0 * hr * hr
    idx = np.arange(N)
    R[idx[:-1] + 1, idx[:-1]] += c
    R[idx[:-1], idx[:-1]] -= c
    R[idx[1:], idx[1:]] += c
    R[idx[1:] - 1, idx[1:]] -= c
    return R @ A_inv


def _build_m1(raw_index, W1):
    """M1 = E @ W1 (N x HID) in f64; E from Hermite weights at t=sigmoid."""
    t = 1.0 / (1.0 + np.exp(-raw_index.astype(np.float64)))
    tn = t * (N - 1)
    idx = np.clip(np.floor(tn), 0, N - 2).astype(np.int64)
    u = tn - idx
    c1 = u * u * (3.0 - 2.0 * u)
    c0 = 1.0 - c1
    c2 = H * u * (u - 1.0) ** 2
    c3 = H * u * u * (u - 1.0)
    K = _k_matrix()
    E = K[:, idx] * c2[None, :] + K[:, idx + 1] * c3[None, :]
    E[idx, np.arange(T)] += c0
    E[idx + 1, np.arange(T)] += c1
    return E @ W1.astype(np.float64)


def _pack_m1(M1p, W2, W3, b1p, b2, b3):
    """Two [128, *] bf16 blocks: chunk-blocked permuted M1, tail + W2|W3.

    Biases ride ones-rows: the last chunk gets two extra rows holding
    b1 in hi+lo bf16 halves (rhs rows are host-packed ones); W2/W3 get
    the same two-row treatment against ones-rows in h1e/h2e.
    """
    import ml_dtypes

    bf = ml_dtypes.bfloat16

    def hi_lo(v):
        hi = v.astype(bf).astype(np.float64)
        return hi.astype(bf), (v - hi).astype(bf)

    P = np.zeros((128, MW + 11), bf)
    for k in range(KT):
        o, rows = COFF[k], CHUNKS[k]
        P[:rows, HID * k : HID * k + HID] = M1p[o : o + rows]
    lr = KT - 1
    rows = CHUNKS[lr]
    b1h, b1l = hi_lo(b1p)
    P[rows, HID * lr : HID * lr + HID] = b1h
    P[rows + 1, HID * lr : HID * lr + HID] = b1l
    P[:HID, MW : MW + HID2] = W2
    b2h, b2l = hi_lo(b2.astype(np.float64))
    P[HID, MW : MW + HID2] = b2h
    P[HID + 1, MW : MW + HID2] = b2l
    P[:HID2, MW + HID2] = W3[:, 0]
    b3h, b3l = hi_lo(b3.astype(np.float64))
    P[HID2, MW + HID2] = b3h[0]
    P[HID2 + 1, MW + HID2] = b3l[0]
    return P[:, : HID * M1SPLIT].copy(), P[:, HID * M1SPLIT :].copy()


# ----------------------------------------------------------------- bass graph
@functools.lru_cache(maxsize=1)
def _build_nc():
    from contextlib import ExitStack

    from concourse import bacc, tile, mybir

    f32 = mybir.dt.float32
    bf16 = mybir.dt.bfloat16
    f8 = mybir.dt.float8e4
    Id = mybir.ActivationFunctionType.Identity
    Lrelu = mybir.ActivationFunctionType.Lrelu

    nc = bacc.Bacc(None, num_devices=NCORES, num_swdge_queues=1)

    xg_d = [
        nc.declare_dram_parameter(
            f"xg{g}", [GROWS[g], n * BPC], f8 if GF8[g] else bf16,
            isOutput=False,
        )
        for g, (_, n) in enumerate(GROUPS)
    ]
    m1h_d = nc.declare_dram_parameter(
        "m1h", [128, HID * M1SPLIT], bf16, isOutput=False
    )
    m1t_d = nc.declare_dram_parameter(
        "m1t", [128, MW + 11 - HID * M1SPLIT], bf16, isOutput=False
    )
    out = nc.declare_dram_parameter("out", [BPC], f32, isOutput=True)

    ctx = ExitStack()
    with ctx:
        tc = ctx.enter_context(tile.TileContext(nc))
        sb = ctx.enter_context(tc.tile_pool(name="sb", bufs=1))
        ps = ctx.enter_context(tc.tile_pool(name="ps", bufs=1, space="PSUM"))

        def stile(shape, dtype, tag):
            return sb.tile(shape, dtype, tag=tag, name=tag)

        # ---- PE clock warm-up: dummy matmuls while DMAs start up
        dmy = stile([128, 512], bf16, "dmy")
        nc.vector.memset(dmy[:], 0.0)
        dps = ps.tile([HID, 512], f32, tag="dps", name="dps")
        for _ in range(6):
            nc.tensor.matmul(
                dps[:], lhsT=dmy[:, 0:HID], rhs=dmy[:], start=True, stop=True
            )

        # ---- DMA issue: queue 0 = sync, queue 1 = scalar
        eng = [nc.sync, nc.scalar]
        m1h = stile([128, HID * M1SPLIT], bf16, "m1h")
        m1t = stile([128, MW + 11 - HID * M1SPLIT], bf16, "m1t")
        eng[1].dma_start(out=m1h[:], in_=m1h_d[:, :])

        xg = []
        for g, (c0_, n) in enumerate(GROUPS):
            xt = stile([GROWS[g], n * BPC], f8 if GF8[g] else bf16, f"xg{g}")
            eng[GQ[g]].dma_start(out=xt[:], in_=xg_d[g][:, :])
            xg.append(xt)
            if g == 0:
                eng[0].dma_start(out=m1t[:], in_=m1t_d[:, :])
        w2s = m1t[0 : HID + 2, MW - HID * M1SPLIT : MW - HID * M1SPLIT + HID2]
        w3s = m1t[0 : HID2 + 2, MW + HID2 - HID * M1SPLIT : MW + HID2 - HID * M1SPLIT + 1]

        def m1_chunk(k, rows):
            if k < M1SPLIT:
                return m1h[0:rows, HID * k : HID * k + HID]
            o = HID * (k - M1SPLIT)
            return m1t[0:rows, o : o + HID]

        # ---- main matmul: h1ps[nh] (HID x 512) += M1_k^T @ xT_k
        h1ps = [
            ps.tile([HID, 512], f32, tag=f"h1ps{nh}", name=f"h1ps{nh}")
            for nh in range(2)
        ]
        for g, (kc0, n) in enumerate(GROUPS):
            if g in (2, 3, 4):
                # keep the PE clock hot across the expected head-DMA gaps
                for _ in range(2):
                    nc.tensor.matmul(
                        dps[:], lhsT=dmy[:, 0:HID], rhs=dmy[:],
                        start=True, stop=True,
                    )
            for j in range(n):
                k = kc0 + j
                # last chunk carries two extra ones-rows bearing b1 hi+lo
                rows = CHUNKS[k] + (2 if k == KT - 1 else 0)
                for nh in range(2):
                    nc.tensor.matmul(
                        h1ps[nh][:],
                        lhsT=m1_chunk(k, rows),
                        rhs=xg[g][0:rows, BPC * j + 512 * nh : BPC * j + 512 * nh + 512],
                        start=(k == 0),
                        stop=(k == KT - 1),
                    )

        # ---- MLP tail: biases ride ones-rows; leaky is one op per half,
        # nh=0 on the scalar engine (Lrelu) and nh=1 on vector (STT max),
        # so the two halves' chains run in parallel.
        op = mybir.AluOpType
        h1e = stile([HID + 2, BPC], bf16, "h1e")
        h2e = stile([HID2 + 2, BPC], bf16, "h2e")
        nc.vector.memset(h1e[:], 1.0)
        nc.vector.memset(h2e[:], 1.0)
        y_sb = stile([1, BPC], f32, "y")
        h2ps = [
            ps.tile([HID2, 512], f32, tag=f"h2ps{nh}", name=f"h2ps{nh}")
            for nh in range(2)
        ]
        yps = [
            ps.tile([1, 512], f32, tag=f"yps{nh}", name=f"yps{nh}")
            for nh in range(2)
        ]
        for nh in range(2):
            sl = slice(512 * nh, 512 * nh + 512)
            nc.scalar.activation(h1e[0:HID, sl], h1ps[nh][:], Lrelu, alpha=0.01)
            nc.tensor.matmul(
                h2ps[nh][:], lhsT=w2s, rhs=h1e[0 : HID + 2, sl],
                start=True, stop=True,
            )
            nc.scalar.activation(h2e[0:HID2, sl], h2ps[nh][:], Lrelu, alpha=0.01)
            nc.tensor.matmul(
                yps[nh][:], lhsT=w3s, rhs=h2e[0 : HID2 + 2, sl],
                start=True, stop=True,
            )
            nc.vector.tensor_copy(out=y_sb[:, sl], in_=yps[nh][:])
            (nc.sync if nh == 0 else nc.scalar).dma_start(
                out=out[:].rearrange("(a b) -> a b", a=1)[:, sl],
                in_=y_sb[:, sl],
            )

    return nc


# ------------------------------------------------------------------- driver
TRACE = False          # set by test harness to capture a profile
LAST_RESULT = None     # BassKernelResults of the last run (when TRACE)


def kernel(x, raw_index, W1, b1, W2, b2, W3, b3):
    global LAST_RESULT
    import ml_dtypes
    from concourse.bass_utils import run_bass_kernel_spmd

    bf = ml_dtypes.bfloat16
    f8 = ml_dtypes.float8_e4m3
    x = np.asarray(x, np.float32)
    M1 = _build_m1(np.asarray(raw_index), np.asarray(W1))
    # bands permuted by |M1| row magnitude: big rows bf16, small rows fp8
    score = (M1 * M1).sum(1)
    order = np.argsort(-score)
    bf_rows = np.sort(order[:NBF])
    f8_rows = np.sort(order[NBF:])
    # assign rows to chunks following the CF8 interleave pattern
    parts, fi, bi = [], 0, 0
    for k in range(KT):
        r = CHUNKS[k]
        if CF8[k]:
            parts.append(f8_rows[fi : fi + r])
            fi += r
        else:
            parts.append(bf_rows[bi : bi + r])
            bi += r
    perm = np.concatenate(parts)
    # fold the fp8 recentering shift (x-0.5) into b1
    b1p = np.asarray(b1, np.float64) + 0.5 * M1[f8_rows].sum(0)
    m1h_a, m1t_a = _pack_m1(
        M1[perm], np.asarray(W2, np.float32), np.asarray(W3, np.float32),
        b1p, np.asarray(b2, np.float32), np.asarray(b3, np.float32),
    )

    nc = _build_nc()
    if not nc.is_finalized():
        nc.finalize()
    in_maps = []
    for p in range(NCORES):
        xs = x[BPC * p : BPC * (p + 1)]  # (BPC, N)
        m = {"m1h": m1h_a, "m1t": m1t_a}
        for g, (kc0, n) in enumerate(GROUPS):
            dt = f8 if GF8[g] else bf
            blk = np.zeros((GROWS[g], n * BPC), dt)
            for j in range(n):
                k = kc0 + j
                o, rows = COFF[k], CHUNKS[k]
                sub = xs[:, perm[o : o + rows]].T  # (rows, BPC)
                if GF8[g]:
                    sub = sub - 0.5
                blk[:rows, BPC * j : BPC * (j + 1)] = sub.astype(dt)
                if k == KT - 1:
                    # ones-rows carrying b1 hi+lo in the matching lhsT rows
                    blk[rows : rows + 2, BPC * j : BPC * (j + 1)] = 1.0
            m[f"xg{g}"] = blk
        in_maps.append(m)
    res = run_bass_kernel_spmd(
        nc, in_maps, core_ids=list(range(NCORES)), trace=TRACE
    )
    if TRACE:
        LAST_RESULT = res
    return np.concatenate(
        [np.asarray(res.results[p]["out"]).ravel() for p in range(NCORES)]
    )


# revision 34
# speedup vs baseline: 1.1156x; 1.1156x over previous
"""Trainium2 Bass kernel for the batched natural-cubic-spline + MLP model.

Math: the whole spline pipeline (natural-cubic coeffs via the constant
tridiagonal solve, evaluation at t = sigmoid(raw_index)) is linear in x:
    outputs = x @ E,   E (N x T),  col j = c0*e_i + c1*e_{i+1}
                                          + c2*K[:,i] + c3*K[:,i+1]
with K = R @ inv(Tridiag) input-independent. E depends only on raw_index,
so M1 = E @ W1 (N x 50) is computed ON HOST in f64 and shipped as bf16.

Device work per core (pure data-parallel, batch split 8 ways):
    h1 = leaky(x @ M1_bf16 + b1')    # 17 k-chunk matmuls x 2 col halves
    h2 = leaky(h1 @ W2 + b2); y = h2 @ W3 + b3
The kernel is HBM-bound, so x ships in mixed precision: the NBF bands
with the largest |M1| rows (biggest error impact) as bf16, the rest as
fp8 e4m3 recentered to x-0.5 (uniform [0,1) data => half the quant
error); the 0.5 shift is folded into b1' on host. Bands are permuted by
magnitude on host; contraction order is irrelevant to the matmul.
x^T streams over both HWDGE queues in host-packed GROUP tiles (small
head groups for an early PE start, big middle groups to amortize the
~0.7us inter-DMA queue bubble). Dummy matmuls at kernel start keep the
PE busy so its DVFS clock is ramped (2.4GHz) when the real stream begins.
Biases ride the scalar-engine activations (bias=), leaky via Lrelu.
"""

import functools

import numpy as np

N = 2000          # bands (spline knots)
T = 500           # eval points
BATCH = 8192
NCORES = 8
BPC = BATCH // NCORES      # 1024 batch rows per core
HID = 50
HID2 = 10
H = 1.0 / (N - 1)
NBF = 384         # bands kept in bf16 (largest |M1| rows)
# contraction chunks in PE order: tiny fp8 head (earliest PE start), then
# uniform ~256KB groups strictly alternating queue and interleaving the
# bf16 singles between fp8 pairs, so consumption tracks both queues
CHUNKS = [32, 96, 128, 128] + [128] + [128, 128] + [128] + [128, 128] + [128] + [128] * 4 + [128, 80]
CF8 = [True] * 4 + [False] + [True, True] + [False] + [True, True] + [False] + [True] * 6
COFF = np.concatenate([[0], np.cumsum(CHUNKS)]).astype(int)
KT = len(CHUNKS)  # 17
MW = KT * HID     # m1 packed width (850)
# x^T DMA groups: (first chunk, n chunks)
GROUPS = [
    (0, 1), (1, 1), (2, 2), (4, 1), (5, 2), (7, 1),
    (8, 2), (10, 1), (11, 2), (13, 2), (15, 2),
]
GROWS = [32, 96, 128, 128, 128, 128, 128, 128, 128, 128, 128]
GF8 = [True, True, True, False, True, False, True, False, True, True, True]
# queue per group (0 = sync, 1 = scalar), alternating in chunk order
GQ = [0, 1, 0, 1, 0, 1, 0, 1, 0, 1, 0]
M1SPLIT = 4       # m1 head covers chunks [0, M1SPLIT), tail the rest + W2/W3


# ----------------------------------------------------------------- host math
@functools.lru_cache(maxsize=1)
def _k_matrix():
    """K (N x N) f64 with kd = x @ K (knot derivatives)."""
    hr = float(N - 1)
    main = np.full(N, 4.0 * hr)
    main[0] = main[-1] = 2.0 * hr
    off = np.full(N - 1, hr)
    A = np.diag(main) + np.diag(off, 1) + np.diag(off, -1)
    A_inv = np.linalg.inv(A)
    R = np.zeros((N, N))
    c = 3.0 * hr * hr
    idx = np.arange(N)
    R[idx[:-1] + 1, idx[:-1]] += c
    R[idx[:-1], idx[:-1]] -= c
    R[idx[1:], idx[1:]] += c
    R[idx[1:] - 1, idx[1:]] -= c
    return R @ A_inv


def _build_m1(raw_index, W1):
    """M1 = E @ W1 (N x HID) in f64; E from Hermite weights at t=sigmoid."""
    t = 1.0 / (1.0 + np.exp(-raw_index.astype(np.float64)))
    tn = t * (N - 1)
    idx = np.clip(np.floor(tn), 0, N - 2).astype(np.int64)
    u = tn - idx
    c1 = u * u * (3.0 - 2.0 * u)
    c0 = 1.0 - c1
    c2 = H * u * (u - 1.0) ** 2
    c3 = H * u * u * (u - 1.0)
    K = _k_matrix()
    E = K[:, idx] * c2[None, :] + K[:, idx + 1] * c3[None, :]
    E[idx, np.arange(T)] += c0
    E[idx + 1, np.arange(T)] += c1
    return E @ W1.astype(np.float64)


def _pack_m1(M1p, W2, W3, b1p, b2, b3):
    """Two [128, *] bf16 blocks: chunk-blocked permuted M1, tail + W2|W3.

    Biases ride ones-rows: the last chunk gets two extra rows holding
    b1 in hi+lo bf16 halves (rhs rows are host-packed ones); W2/W3 get
    the same two-row treatment against ones-rows in h1e/h2e.
    """
    import ml_dtypes

    bf = ml_dtypes.bfloat16

    def hi_lo(v):
        hi = v.astype(bf).astype(np.float64)
        return hi.astype(bf), (v - hi).astype(bf)

    P = np.zeros((128, MW + 11), bf)
    for k in range(KT):
        o, rows = COFF[k], CHUNKS[k]
        P[:rows, HID * k : HID * k + HID] = M1p[o : o + rows]
    lr = KT - 1
    rows = CHUNKS[lr]
    b1h, b1l = hi_lo(b1p)
    P[rows, HID * lr : HID * lr + HID] = b1h
    P[rows + 1, HID * lr : HID * lr + HID] = b1l
    P[:HID, MW : MW + HID2] = W2
    b2h, b2l = hi_lo(b2.astype(np.float64))
    P[HID, MW : MW + HID2] = b2h
    P[HID + 1, MW : MW + HID2] = b2l
    P[:HID2, MW + HID2] = W3[:, 0]
    b3h, b3l = hi_lo(b3.astype(np.float64))
    P[HID2, MW + HID2] = b3h[0]
    P[HID2 + 1, MW + HID2] = b3l[0]
    return P[:, : HID * M1SPLIT].copy(), P[:, HID * M1SPLIT :].copy()


# ----------------------------------------------------------------- bass graph
@functools.lru_cache(maxsize=1)
def _build_nc():
    from contextlib import ExitStack

    from concourse import bacc, tile, mybir

    f32 = mybir.dt.float32
    bf16 = mybir.dt.bfloat16
    f8 = mybir.dt.float8e4
    Id = mybir.ActivationFunctionType.Identity
    Lrelu = mybir.ActivationFunctionType.Lrelu

    nc = bacc.Bacc(None, num_devices=NCORES, num_swdge_queues=1)

    xg_d = [
        nc.declare_dram_parameter(
            f"xg{g}", [GROWS[g], n * BPC], f8 if GF8[g] else bf16,
            isOutput=False,
        )
        for g, (_, n) in enumerate(GROUPS)
    ]
    m1h_d = nc.declare_dram_parameter(
        "m1h", [128, HID * M1SPLIT], bf16, isOutput=False
    )
    m1t_d = nc.declare_dram_parameter(
        "m1t", [128, MW + 11 - HID * M1SPLIT], bf16, isOutput=False
    )
    out = nc.declare_dram_parameter("out", [BPC], f32, isOutput=True)

    ctx = ExitStack()
    with ctx:
        tc = ctx.enter_context(tile.TileContext(nc))
        sb = ctx.enter_context(tc.tile_pool(name="sb", bufs=1))
        ps = ctx.enter_context(tc.tile_pool(name="ps", bufs=1, space="PSUM"))

        def stile(shape, dtype, tag):
            return sb.tile(shape, dtype, tag=tag, name=tag)

        # ---- PE clock warm-up: dummy matmuls while DMAs start up
        dmy = stile([128, 512], bf16, "dmy")
        nc.vector.memset(dmy[:], 0.0)
        dps = ps.tile([HID, 512], f32, tag="dps", name="dps")
        for _ in range(6):
            nc.tensor.matmul(
                dps[:], lhsT=dmy[:, 0:HID], rhs=dmy[:], start=True, stop=True
            )

        # ---- DMA issue: queue 0 = sync, queue 1 = scalar
        eng = [nc.sync, nc.scalar]
        m1h = stile([128, HID * M1SPLIT], bf16, "m1h")
        m1t = stile([128, MW + 11 - HID * M1SPLIT], bf16, "m1t")
        eng[1].dma_start(out=m1h[:], in_=m1h_d[:, :])

        xg = []
        for g, (c0_, n) in enumerate(GROUPS):
            xt = stile([GROWS[g], n * BPC], f8 if GF8[g] else bf16, f"xg{g}")
            eng[GQ[g]].dma_start(out=xt[:], in_=xg_d[g][:, :])
            xg.append(xt)
            if g == 0:
                eng[0].dma_start(out=m1t[:], in_=m1t_d[:, :])
        w2s = m1t[0 : HID + 2, MW - HID * M1SPLIT : MW - HID * M1SPLIT + HID2]
        w3s = m1t[0 : HID2 + 2, MW + HID2 - HID * M1SPLIT : MW + HID2 - HID * M1SPLIT + 1]

        def m1_chunk(k, rows):
            if k < M1SPLIT:
                return m1h[0:rows, HID * k : HID * k + HID]
            o = HID * (k - M1SPLIT)
            return m1t[0:rows, o : o + HID]

        # ---- main matmul: h1ps[nh] (HID x 512) += M1_k^T @ xT_k
        h1ps = [
            ps.tile([HID, 512], f32, tag=f"h1ps{nh}", name=f"h1ps{nh}")
            for nh in range(2)
        ]
        for g, (kc0, n) in enumerate(GROUPS):
            if g in (2, 3, 4):
                # keep the PE clock hot across the expected head-DMA gaps
                for _ in range(2):
                    nc.tensor.matmul(
                        dps[:], lhsT=dmy[:, 0:HID], rhs=dmy[:],
                        start=True, stop=True,
                    )
            for j in range(n):
                k = kc0 + j
                # last chunk carries two extra ones-rows bearing b1 hi+lo
                rows = CHUNKS[k] + (2 if k == KT - 1 else 0)
                for nh in range(2):
                    nc.tensor.matmul(
                        h1ps[nh][:],
                        lhsT=m1_chunk(k, rows),
                        rhs=xg[g][0:rows, BPC * j + 512 * nh : BPC * j + 512 * nh + 512],
                        start=(k == 0),
                        stop=(k == KT - 1),
                    )

        # ---- MLP tail: biases ride ones-rows; leaky is one op per half,
        # nh=0 on the scalar engine (Lrelu) and nh=1 on vector (STT max),
        # so the two halves' chains run in parallel.
        op = mybir.AluOpType
        h1e = stile([HID + 2, BPC], bf16, "h1e")
        h2e = stile([HID2 + 2, BPC], bf16, "h2e")
        nc.vector.memset(h1e[:], 1.0)
        nc.vector.memset(h2e[:], 1.0)
        y_sb = stile([1, BPC], f32, "y")
        h2ps = [
            ps.tile([HID2, 512], f32, tag=f"h2ps{nh}", name=f"h2ps{nh}")
            for nh in range(2)
        ]
        yps = [
            ps.tile([1, 512], f32, tag=f"yps{nh}", name=f"yps{nh}")
            for nh in range(2)
        ]
        for nh in range(2):
            sl = slice(512 * nh, 512 * nh + 512)
            nc.scalar.activation(h1e[0:HID, sl], h1ps[nh][:], Lrelu, alpha=0.01)
            nc.tensor.matmul(
                h2ps[nh][:], lhsT=w2s, rhs=h1e[0 : HID + 2, sl],
                start=True, stop=True,
            )
            nc.scalar.activation(h2e[0:HID2, sl], h2ps[nh][:], Lrelu, alpha=0.01)
            nc.tensor.matmul(
                yps[nh][:], lhsT=w3s, rhs=h2e[0 : HID2 + 2, sl],
                start=True, stop=True,
            )
            nc.vector.tensor_copy(out=y_sb[:, sl], in_=yps[nh][:])
            (nc.sync if nh == 0 else nc.scalar).dma_start(
                out=out[:].rearrange("(a b) -> a b", a=1)[:, sl],
                in_=y_sb[:, sl],
            )

    return nc


# ------------------------------------------------------------------- driver
TRACE = False          # set by test harness to capture a profile
LAST_RESULT = None     # BassKernelResults of the last run (when TRACE)


def kernel(x, raw_index, W1, b1, W2, b2, W3, b3):
    global LAST_RESULT
    import ml_dtypes
    from concourse.bass_utils import run_bass_kernel_spmd

    bf = ml_dtypes.bfloat16
    f8 = ml_dtypes.float8_e4m3
    x = np.asarray(x, np.float32)
    M1 = _build_m1(np.asarray(raw_index), np.asarray(W1))
    # bands permuted by |M1| row magnitude: big rows bf16, small rows fp8
    score = (M1 * M1).sum(1)
    order = np.argsort(-score)
    bf_rows = np.sort(order[:NBF])
    f8_rows = np.sort(order[NBF:])
    # assign rows to chunks following the CF8 interleave pattern
    parts, fi, bi = [], 0, 0
    for k in range(KT):
        r = CHUNKS[k]
        if CF8[k]:
            parts.append(f8_rows[fi : fi + r])
            fi += r
        else:
            parts.append(bf_rows[bi : bi + r])
            bi += r
    perm = np.concatenate(parts)
    # fold the fp8 recentering shift (x-0.5) into b1
    b1p = np.asarray(b1, np.float64) + 0.5 * M1[f8_rows].sum(0)
    m1h_a, m1t_a = _pack_m1(
        M1[perm], np.asarray(W2, np.float32), np.asarray(W3, np.float32),
        b1p, np.asarray(b2, np.float32), np.asarray(b3, np.float32),
    )

    nc = _build_nc()
    if not nc.is_finalized():
        nc.finalize()
    in_maps = []
    for p in range(NCORES):
        xs = x[BPC * p : BPC * (p + 1)]  # (BPC, N)
        m = {"m1h": m1h_a, "m1t": m1t_a}
        for g, (kc0, n) in enumerate(GROUPS):
            dt = f8 if GF8[g] else bf
            blk = np.zeros((GROWS[g], n * BPC), dt)
            for j in range(n):
                k = kc0 + j
                o, rows = COFF[k], CHUNKS[k]
                sub = xs[:, perm[o : o + rows]].T  # (rows, BPC)
                if GF8[g]:
                    sub = sub - 0.5
                blk[:rows, BPC * j : BPC * (j + 1)] = sub.astype(dt)
                if k == KT - 1:
                    # ones-rows carrying b1 hi+lo in the matching lhsT rows
                    blk[rows : rows + 2, BPC * j : BPC * (j + 1)] = 1.0
            m[f"xg{g}"] = blk
        in_maps.append(m)
    res = run_bass_kernel_spmd(
        nc, in_maps, core_ids=list(range(NCORES)), trace=TRACE
    )
    if TRACE:
        LAST_RESULT = res
    return np.concatenate(
        [np.asarray(res.results[p]["out"]).ravel() for p in range(NCORES)]
    )
